# revision 6
# baseline (speedup 1.0000x reference)
"""Trainium2 Bass kernel: Conv2d(1->64, k=7, valid) on data [32,1,224,224] f32.

Data-parallel over batch (4 images per core on 8 cores).  Per core:
im2col matmul in fp16 (fp32 PSUM, K=49).  Design is driven by two
measured hardware limits: per-core HBM bandwidth (~255 GB/s combined
read+write across the 16 DMA engines) and the PE duty-cycle throttle
(sustained rate ~2.4 cols/ns with two concurrent 64-col row groups).

HBM traffic is minimized: input is read once as 7 ky-shifted slab
copies (4.1 MB), the 49-way im2col replication happens SBUF->SBUF
(doesn't touch HBM), and the output leaves as fp16 with the 6 garbage
columns stripped (24.3 MB written).

Layout/pipeline (per core, 16 row-block "tiles" of 56 output rows,
processed as 8 pairs):
  - host: for each tile, SEVEN slab copies (slab ky = flat image
    elements [(r0+ky)*224 : +RUN]).  Tile t's slabs sit at partitions
    base+4*ky (cycling all 4 SBUF ports), base = (t//2)%4 + 64*(t%2),
    free slot (t//2)//4; loaded tile-by-tile on the sync HWDGE queue.
  - im2col: ONE SWDGE DMA per tile (3-dim AP): dim0 walks the 7 slab
    partitions, dim1 the 7 kx shifts (stride-1 overlapping reads),
    dim2 a contiguous 25KB run.  dst = [49, RUN] at partition 0 (tile
    A, PE row group h0) or 64 (tile B, h1).
  - matmul: per 448-col PSUM chunk, 2 concurrent matmuls: A -> ps[0:64]
    and B -> ps[64:128] (different row groups dual-issue on the PE).
  - copy: psum [128,448] f32 (= 2 output rows of 224) -> ob fp16
    keeping only cols 0..217 of each row, alternating DVE/ACT.
  - out: fp16 stores (no cast) on the sync HWDGE queue, one DMA per
    tile [64ch, nrows*218] (24KB descriptors).  Host only casts
    fp16->fp32 (lossless).
"""

import numpy as np

B = 32            # full batch
OC = 64           # out channels
KS = 7            # kernel size
H = 224           # input H=W
OH = 218          # valid output rows/cols
OW = 224          # im2col row width (incl 6 garbage cols)
NCORES = 8
IPC = B // NCORES  # images per core

BLK = 56          # output rows per tile
NBLK = 4          # tiles per image (3x56 + 1x50 valid rows)
NTILES = IPC * NBLK
NPAIRS = NTILES // 2
NCOLS = BLK * OW  # 12544 im2col columns per tile
RUN = NCOLS + 8   # slab length / per-partition run (covers kx shifts)
CHUNK = 448       # psum chunk columns (= 2 output rows)
NCHUNK = NCOLS // CHUNK  # 28
OBW = CHUNK // OW * OH   # 436 ob columns per chunk (garbage stripped)

KP = KS * KS      # 49 im2col partitions per tile

_CACHE = {}


def _tile_src(t):
    q = t // 2
    base = (q % 4) + 64 * (t % 2)
    return base, q // 4  # partition base, free slot


def _build():
    import concourse.bass as bass
    import concourse.mybir as mybir
    import concourse.tile as tile
    from concourse import bacc

    nc = bacc.Bacc("TRN2", target_bir_lowering=False, debug=False)

    xb = nc.dram_tensor("xb", [NTILES, KS, RUN], mybir.dt.float16,
                        kind="ExternalInput")
    wbd = nc.dram_tensor("wbd", [2, KP, OC], mybir.dt.float16,
                         kind="ExternalInput")
    out = nc.dram_tensor("out", [IPC, OC, OH, OH], mybir.dt.float16,
                         kind="ExternalOutput")

    with tile.TileContext(nc) as tc:
        with (
            tc.tile_pool(name="src", bufs=1) as src_pool,
            tc.tile_pool(name="wp", bufs=1) as w_pool,
            tc.tile_pool(name="i2c", bufs=3) as i2c_pool,
            tc.tile_pool(name="ob", bufs=2) as ob_pool,
            tc.tile_pool(name="ps", bufs=8, space="PSUM") as ps_pool,
        ):
            srct = src_pool.tile([128, 2 * RUN], mybir.dt.float16)
            p_stride = srct.ap[0][0]  # partition pitch in elements

            wt = w_pool.tile([128, OC], mybir.dt.float16)
            nc.scalar.dma_start(out=wt[0:KP, :], in_=wbd[0, :, :])
            nc.scalar.dma_start(out=wt[64:64 + KP, :], in_=wbd[1, :, :])

            # per-tile slab loads (HWDGE), in consumption order
            for t in range(NTILES):
                base, slot = _tile_src(t)
                dst = bass.AP(
                    tensor=srct.tensor,
                    offset=srct.offset + base * p_stride + slot * RUN,
                    ap=[[4 * p_stride, KS], [1, RUN]],
                )
                nc.sync.dma_start(out=dst, in_=xb[t, :, :])

            for q in range(NPAIRS):
                i2c = i2c_pool.tile([128, RUN], mybir.dt.float16,
                                    tag="i2c", name=f"i2c{q}")
                for half in range(2):
                    t = 2 * q + half
                    base, slot = _tile_src(t)
                    src = bass.AP(
                        tensor=srct.tensor,
                        offset=srct.offset + base * p_stride + slot * RUN,
                        ap=[[4 * p_stride, KS], [1, KS], [1, RUN]],
                    )
                    nc.gpsimd.dma_start(
                        out=i2c[64 * half:64 * half + KP, :], in_=src)

                ob = ob_pool.tile([128, NCHUNK * OBW], mybir.dt.float16,
                                  tag="ob")
                for j in range(NCHUNK):
                    ps = ps_pool.tile([128, CHUNK], mybir.dt.float32,
                                      tag="ps")
                    c0 = CHUNK * j
                    nc.tensor.matmul(
                        ps[0:OC, :], wt[0:KP, :],
                        i2c[0:KP, c0:c0 + CHUNK],
                        start=True, stop=True)
                    nc.tensor.matmul(
                        ps[OC:128, :], wt[64:64 + KP, :],
                        i2c[64:64 + KP, c0:c0 + CHUNK],
                        start=True, stop=True)
                    # strip the 6 garbage cols of each 224-col output row
                    pssrc = bass.AP(
                        tensor=ps.tensor, offset=ps.offset,
                        ap=[[ps.ap[0][0], 128], [OW, CHUNK // OW], [1, OH]],
                    )
                    if j % 2 == 0:
                        nc.vector.tensor_copy(
                            ob[:, OBW * j:OBW * (j + 1)], pssrc)
                    else:
                        nc.scalar.copy(
                            ob[:, OBW * j:OBW * (j + 1)], pssrc)

                for half in range(2):
                    t = 2 * q + half
                    imgi, blk = divmod(t, NBLK)
                    r0 = BLK * blk
                    nrows = min(BLK, OH - r0)
                    nc.sync.dma_start(
                        out=out[imgi, :, r0:r0 + nrows, :],
                        in_=ob[64 * half:64 * half + OC, :nrows * OH])

    nc.compile()
    return nc


def _prep_inputs(data, weight):
    d = np.asarray(data).reshape(B, H, H).astype(np.float16)
    dpad = np.zeros((B, 256, H), dtype=np.float16)
    dpad[:, :H, :] = d
    dflat = dpad.reshape(B, 256 * H)
    w = np.asarray(weight).reshape(OC, KS * KS).astype(np.float16)

    wbd = np.empty((2, KP, OC), dtype=np.float16)
    wbd[0] = w.T
    wbd[1] = w.T

    in_maps = []
    for c in range(NCORES):
        xb = np.empty((NTILES, KS, RUN), dtype=np.float16)
        for t in range(NTILES):
            imgi, blk = divmod(t, NBLK)
            g = c * IPC + imgi
            r0 = BLK * blk
            for ky in range(KS):
                base = (r0 + ky) * H
                xb[t, ky, :] = dflat[g, base:base + RUN]
        in_maps.append({"xb": xb, "wbd": wbd})
    return in_maps


def kernel(data, weight):
    from concourse.bass_utils import run_bass_kernel_spmd

    if "nc" not in _CACHE:
        _CACHE["nc"] = _build()
    nc = _CACHE["nc"]

    in_maps = _prep_inputs(np.asarray(data), np.asarray(weight))
    res = run_bass_kernel_spmd(nc, in_maps, core_ids=list(range(NCORES)))
    outs = [r["out"] for r in res.results]
    full = np.concatenate(outs, axis=0)  # [32, 64, 218, 218] f16
    return full.astype(np.float32)


# revision 7
# speedup vs baseline: 1.0073x; 1.0073x over previous
"""Trainium2 Bass kernel: Conv2d(1->64, k=7, valid) on data [32,1,224,224] f32.

Data-parallel over batch (4 images per core on 8 cores).  Per core:
im2col matmul in fp16 (fp32 PSUM).  Two measured hardware limits drive
the design: the 16 DMA engines sustain ~264 GB/s of aggregate READ
traffic (SBUF or HBM alike), and the PE duty-cycle throttle caps
sustained streaming at ~2.4 cols/ns (two concurrent 64-col row groups
at 1.2 GHz effective).

The kernel balances the two: most tile-pairs materialize only 4 of the
7 kx shifts in the im2col (fewer DMA bytes, but 2 PSUM-accumulating
matmuls per chunk = 2x tensor time); 2 of the 8 pairs materialize all
7 (full-rate single matmul).  Outputs leave as fp16 with the 6 garbage
columns stripped during the PSUM->SBUF copies.

Layout/pipeline (per core, 16 row-block "tiles" of 56 output rows,
processed as 8 pairs):
  - host: builds the partial im2col per tile in DRAM: KP rows (7 ky x
    KXL kx shifts), each a contiguous run of 56*224+8 fp16 elements.
  - i2c: one contiguous [KP, RUN] DMA per tile from DRAM.  Pair
    layout: tile A at partitions 0.. (PE row group h0), B at 64.. (h1).
  - matmul: per 448-col PSUM chunk, NMAT matmuls per tile half, halves
    dual-issued on the PE; matmul m reads the rhs at free-dim offset
    m*KXL (zero weight rows pad kx=7).
  - copy: psum [128,448] f32 (= 2 output rows of 224) -> ob fp16
    keeping only cols 0..217 of each row, alternating DVE/ACT.
  - out: fp16 stores (no cast) on the sync HWDGE queue, one DMA per
    tile [64ch, nrows*218].  Host only casts fp16->fp32 (lossless).
"""

import numpy as np

B = 32            # full batch
OC = 64           # out channels
KS = 7            # kernel size
H = 224           # input H=W
OH = 218          # valid output rows/cols
OW = 224          # im2col row width (incl 6 garbage cols)
NCORES = 8
IPC = B // NCORES  # images per core

BLK = 56          # output rows per tile
NBLK = 4          # tiles per image (3x56 + 1x50 valid rows)
NTILES = IPC * NBLK
NPAIRS = NTILES // 2
NCOLS = BLK * OW  # 12544 im2col columns per tile
RUN = NCOLS + 8   # per-partition run (covers kx shifts)
CHUNK = 448       # psum chunk columns (= 2 output rows)
NCHUNK = NCOLS // CHUNK  # 28
OBW = CHUNK // OW * OH   # 436 ob columns per chunk (garbage stripped)

KPMAX = KS * KS   # 49 im2col partitions for a full (KXL=7) tile

# per-pair kx materialization: 4 -> [28,RUN] i2c + 2 matmuls/chunk,
# 7 -> [49,RUN] i2c + 1 matmul/chunk.  6:2 mix balances the ~264 GB/s
# DMA-read limit against the ~2.4 col/ns sustained PE limit.
PAIR_KXL = [4, 4, 7, 4, 4, 7, 4, 4]

_CACHE = {}


def _build():
    import concourse.mybir as mybir
    import concourse.tile as tile
    from concourse import bacc

    nc = bacc.Bacc("TRN2", target_bir_lowering=False, debug=False)

    i2cd = nc.dram_tensor("i2cd", [NTILES, KPMAX, RUN], mybir.dt.float16,
                          kind="ExternalInput")
    # wbd[g, :, 0:64] = KXL7 weights; [g, :, 64+64m : 128+64m] = KXL4 m-th
    wbd = nc.dram_tensor("wbd", [2, KPMAX, 3 * OC], mybir.dt.float16,
                         kind="ExternalInput")
    out = nc.dram_tensor("out", [IPC, OC, OH, OH], mybir.dt.float16,
                         kind="ExternalOutput")

    with tile.TileContext(nc) as tc:
        with (
            tc.tile_pool(name="wp", bufs=1) as w_pool,
            tc.tile_pool(name="i2c", bufs=3) as i2c_pool,
            tc.tile_pool(name="ob", bufs=3) as ob_pool,
            tc.tile_pool(name="ps", bufs=8, space="PSUM") as ps_pool,
        ):
            import concourse.bass as bass

            wt = w_pool.tile([128, 3 * OC], mybir.dt.float16)
            nc.scalar.dma_start(out=wt[0:KPMAX, :], in_=wbd[0, :, :])
            nc.scalar.dma_start(out=wt[64:64 + KPMAX, :], in_=wbd[1, :, :])

            for q in range(NPAIRS):
                kxl = PAIR_KXL[q]
                nmat = -(-KS // kxl)
                kp = KS * kxl

                i2c = i2c_pool.tile([128, RUN], mybir.dt.float16,
                                    tag="i2c", name=f"i2c{q}")
                for half in range(2):
                    nc.gpsimd.dma_start(
                        out=i2c[64 * half:64 * half + kp, :],
                        in_=i2cd[2 * q + half, 0:kp, :])

                ob = ob_pool.tile([128, NCHUNK * OBW], mybir.dt.float16,
                                  tag="ob")
                for j in range(NCHUNK):
                    ps = ps_pool.tile([128, CHUNK], mybir.dt.float32,
                                      tag="ps")
                    c0 = CHUNK * j
                    for m in range(nmat):
                        st, sp = (m == 0), (m == nmat - 1)
                        wc = 0 if kxl == KS else OC * (1 + m)
                        nc.tensor.matmul(
                            ps[0:OC, :], wt[0:kp, wc:wc + OC],
                            i2c[0:kp, c0 + m * kxl:c0 + m * kxl + CHUNK],
                            start=st, stop=sp)
                        nc.tensor.matmul(
                            ps[OC:128, :], wt[64:64 + kp, wc:wc + OC],
                            i2c[64:64 + kp, c0 + m * kxl:c0 + m * kxl + CHUNK],
                            start=st, stop=sp)
                    # strip the 6 garbage cols of each 224-col output row
                    pssrc = bass.AP(
                        tensor=ps.tensor, offset=ps.offset,
                        ap=[[ps.ap[0][0], 128], [OW, CHUNK // OW], [1, OH]],
                    )
                    if j % 2 == 0:
                        nc.vector.tensor_copy(
                            ob[:, OBW * j:OBW * (j + 1)], pssrc)
                    else:
                        nc.scalar.copy(ob[:, OBW * j:OBW * (j + 1)], pssrc)

                for half in range(2):
                    t = 2 * q + half
                    imgi, blk = divmod(t, NBLK)
                    r0 = BLK * blk
                    nrows = min(BLK, OH - r0)
                    nc.sync.dma_start(
                        out=out[imgi, :, r0:r0 + nrows, :],
                        in_=ob[64 * half:64 * half + OC, :nrows * OH])

    nc.compile()
    return nc


def _prep_inputs(data, weight):
    d = np.asarray(data).reshape(B, H, H).astype(np.float16)
    dpad = np.zeros((B, 256, H), dtype=np.float16)
    dpad[:, :H, :] = d
    dflat = dpad.reshape(B, 256 * H)
    w = np.asarray(weight).reshape(OC, KS, KS).astype(np.float16)

    wbd = np.zeros((2, KPMAX, 3 * OC), dtype=np.float16)
    w7 = w.reshape(OC, KS * KS).T              # [49, 64]
    wbd[:, :, 0:OC] = w7
    for m in range(2):
        for ky in range(KS):
            for kxl in range(4):
                kx = m * 4 + kxl
                if kx >= KS:
                    continue
                wbd[:, ky * 4 + kxl, OC * (1 + m):OC * (2 + m)] = w[:, ky, kx]

    in_maps = []
    for c in range(NCORES):
        i2cd = np.zeros((NTILES, KPMAX, RUN), dtype=np.float16)
        for t in range(NTILES):
            imgi, blk = divmod(t, NBLK)
            g = c * IPC + imgi
            r0 = BLK * blk
            kxl = PAIR_KXL[t // 2]
            for ky in range(KS):
                base = (r0 + ky) * H
                for kxi in range(kxl):
                    i2cd[t, ky * kxl + kxi, :] = \
                        dflat[g, base + kxi:base + kxi + RUN]
        in_maps.append({"i2cd": i2cd, "wbd": wbd})
    return in_maps


def kernel(data, weight):
    from concourse.bass_utils import run_bass_kernel_spmd

    if "nc" not in _CACHE:
        _CACHE["nc"] = _build()
    nc = _CACHE["nc"]

    in_maps = _prep_inputs(np.asarray(data), np.asarray(weight))
    res = run_bass_kernel_spmd(nc, in_maps, core_ids=list(range(NCORES)))
    outs = [r["out"] for r in res.results]
    full = np.concatenate(outs, axis=0)  # [32, 64, 218, 218] f16
    return full.astype(np.float32)


# revision 8
# speedup vs baseline: 1.4741x; 1.4635x over previous
"""Trainium2 Bass kernel: Conv2d(1->64, k=7, valid) on data [32,1,224,224] f32.

Data-parallel over batch (4 images per core on 8 cores).  Per core:
im2col matmul in fp16 (fp32 PSUM).  Two measured hardware limits drive
the design: the 16 DMA engines sustain ~264 GB/s of aggregate READ
traffic (SBUF or HBM alike), and the PE duty-cycle throttle caps
sustained streaming at ~2.4 cols/ns (two concurrent 64-col row groups
at 1.2 GHz effective).

The kernel balances the two: 12 of 16 tile-pairs materialize only 4 of
the 7 kx shifts in the im2col (fewer DMA bytes, but 2 PSUM-accumulating
matmuls per chunk = 2x tensor time); 4 pairs materialize all 7
(full-rate single matmul, more DMA).  Outputs leave as fp16.

Layout/pipeline (per core, 32 row-block "tiles" of 28 output rows,
processed as 16 pairs):
  - host: builds the partial im2col per tile in DRAM: KP rows (7 ky x
    KXL kx shifts), each a contiguous run of 28*224+8 fp16 elements.
  - i2c: one contiguous [KP, RUN] DMA per tile from DRAM.  Pair
    layout: tile A at partitions 0.. (PE row group h0), B at 64.. (h1).
  - matmul: per 448-col PSUM chunk, NMAT matmuls per tile half, halves
    dual-issued on the PE; matmul m reads the rhs at free-dim offset
    m*KXL (zero weight rows pad kx=7).
  - copy: psum [128,448] f32 -> ob fp16, alternating DVE/ACT.
  - out: fp16 stores (no cast) on the sync HWDGE queue, one DMA per
    tile [64ch, nrows*224].  Cols 218..223 are garbage (kx wrap) and
    are sliced off on the host, which also does the lossless
    fp16->fp32 cast of the result.
"""

import numpy as np

B = 32            # full batch
OC = 64           # out channels
KS = 7            # kernel size
H = 224           # input H=W
OH = 218          # valid output rows/cols
OW = 224          # im2col row width (incl 6 garbage cols)
NCORES = 8
IPC = B // NCORES  # images per core

BLK = 28          # output rows per tile
NBLK = 8          # tiles per image (7x28 + 1x22 valid rows)
NTILES = IPC * NBLK
NPAIRS = NTILES // 2
NCOLS = BLK * OW  # 6272 im2col columns per tile
RUN = NCOLS + 8   # per-partition run (covers kx shifts)
CHUNK = 448       # psum chunk columns
NCHUNK = NCOLS // CHUNK  # 14

KPMAX = KS * KS   # 49 im2col partitions for a full (KXL=7) tile

# per-pair kx materialization: 4 -> [28,RUN] i2c + 2 matmuls/chunk,
# 7 -> [49,RUN] i2c + 1 matmul/chunk.  12:4 mix balances the ~264 GB/s
# DMA-read limit against the ~2.4 col/ns sustained PE limit.
PAIR_KXL = [4, 4, 7, 4, 4, 4, 7, 4, 4, 4, 7, 4, 4, 4, 7, 4]

_CACHE = {}


def _build():
    import concourse.mybir as mybir
    import concourse.tile as tile
    from concourse import bacc

    nc = bacc.Bacc("TRN2", target_bir_lowering=False, debug=False)

    i2cd = nc.dram_tensor("i2cd", [NTILES, KPMAX, RUN], mybir.dt.float16,
                          kind="ExternalInput")
    # wbd[g, :, 0:64] = KXL7 weights; [g, :, 64+64m : 128+64m] = KXL4 m-th
    wbd = nc.dram_tensor("wbd", [2, KPMAX, 3 * OC], mybir.dt.float16,
                         kind="ExternalInput")
    out = nc.dram_tensor("out", [IPC, OC, OH, OW], mybir.dt.float16,
                         kind="ExternalOutput")

    with tile.TileContext(nc) as tc:
        with (
            tc.tile_pool(name="wp", bufs=1) as w_pool,
            tc.tile_pool(name="i2c", bufs=6) as i2c_pool,
            tc.tile_pool(name="ob", bufs=4) as ob_pool,
            tc.tile_pool(name="ps", bufs=8, space="PSUM") as ps_pool,
        ):
            wt = w_pool.tile([128, 3 * OC], mybir.dt.float16)
            nc.scalar.dma_start(out=wt[0:KPMAX, :], in_=wbd[0, :, :])
            nc.scalar.dma_start(out=wt[64:64 + KPMAX, :], in_=wbd[1, :, :])

            for q in range(NPAIRS):
                kxl = PAIR_KXL[q]
                nmat = -(-KS // kxl)
                kp = KS * kxl

                i2c = i2c_pool.tile([128, RUN], mybir.dt.float16,
                                    tag="i2c", name=f"i2c{q}")
                for half in range(2):
                    nc.gpsimd.dma_start(
                        out=i2c[64 * half:64 * half + kp, :],
                        in_=i2cd[2 * q + half, 0:kp, :])

                ob = ob_pool.tile([128, NCOLS], mybir.dt.float16, tag="ob")
                for j in range(NCHUNK):
                    ps = ps_pool.tile([128, CHUNK], mybir.dt.float32,
                                      tag="ps")
                    c0 = CHUNK * j
                    for m in range(nmat):
                        st, sp = (m == 0), (m == nmat - 1)
                        wc = 0 if kxl == KS else OC * (1 + m)
                        nc.tensor.matmul(
                            ps[0:OC, :], wt[0:kp, wc:wc + OC],
                            i2c[0:kp, c0 + m * kxl:c0 + m * kxl + CHUNK],
                            start=st, stop=sp)
                        nc.tensor.matmul(
                            ps[OC:128, :], wt[64:64 + kp, wc:wc + OC],
                            i2c[64:64 + kp, c0 + m * kxl:c0 + m * kxl + CHUNK],
                            start=st, stop=sp)
                    if j % 2 == 0:
                        nc.vector.tensor_copy(ob[:, c0:c0 + CHUNK], ps[:, :])
                    else:
                        nc.scalar.copy(ob[:, c0:c0 + CHUNK], ps[:, :])

                for half in range(2):
                    t = 2 * q + half
                    imgi, blk = divmod(t, NBLK)
                    r0 = BLK * blk
                    nrows = min(BLK, OH - r0)
                    nc.sync.dma_start(
                        out=out[imgi, :, r0:r0 + nrows, :],
                        in_=ob[64 * half:64 * half + OC, :nrows * OW])

    nc.compile()
    return nc


def _prep_inputs(data, weight):
    d = np.asarray(data).reshape(B, H, H).astype(np.float16)
    dpad = np.zeros((B, 256, H), dtype=np.float16)
    dpad[:, :H, :] = d
    dflat = dpad.reshape(B, 256 * H)
    w = np.asarray(weight).reshape(OC, KS, KS).astype(np.float16)

    wbd = np.zeros((2, KPMAX, 3 * OC), dtype=np.float16)
    wbd[:, :, 0:OC] = w.reshape(OC, KS * KS).T
    for m in range(2):
        for ky in range(KS):
            for kxl in range(4):
                kx = m * 4 + kxl
                if kx >= KS:
                    continue
                wbd[:, ky * 4 + kxl, OC * (1 + m):OC * (2 + m)] = w[:, ky, kx]

    in_maps = []
    for c in range(NCORES):
        i2cd = np.zeros((NTILES, KPMAX, RUN), dtype=np.float16)
        for t in range(NTILES):
            imgi, blk = divmod(t, NBLK)
            g = c * IPC + imgi
            r0 = BLK * blk
            kxl = PAIR_KXL[t // 2]
            for ky in range(KS):
                base = (r0 + ky) * H
                for kxi in range(kxl):
                    i2cd[t, ky * kxl + kxi, :] = \
                        dflat[g, base + kxi:base + kxi + RUN]
        in_maps.append({"i2cd": i2cd, "wbd": wbd})
    return in_maps


def kernel(data, weight):
    from concourse.bass_utils import run_bass_kernel_spmd

    if "nc" not in _CACHE:
        _CACHE["nc"] = _build()
    nc = _CACHE["nc"]

    in_maps = _prep_inputs(np.asarray(data), np.asarray(weight))
    res = run_bass_kernel_spmd(nc, in_maps, core_ids=list(range(NCORES)))
    outs = [r["out"] for r in res.results]
    full = np.concatenate(outs, axis=0)  # [32, 64, 218, 224] f16
    return np.ascontiguousarray(full[:, :, :, :OH]).astype(np.float32)
